# revision 46
# baseline (speedup 1.0000x reference)
"""BetaGNN message-passing kernel for 8 Trainium2 NeuronCores.

Strategy (1D node partitioning):
  - nodes are sharded across 8 cores (12500 rows/core, padded to 12544 = 98*128)
  - H = relu(X @ W_in + b_in) per-shard feature-major (X, W_in in fp16), transposed on PE
    into a node-major fp16 gather table, AllGathered to all cores
  - SpMM (y[r] = sum vals[e]*H[col[e]] over edges with row[e]==r): edges are
    grouped by (dest-tile x src-quartile) cell with EXACT max-over-cores cell
    sizes (no per-cell 128-rounding); one gather call per (PSUM bank, quartile)
    fetches source rows via the GPSIMD dma_gather custom op (int16 indices over
    4 quarter-tables, 4 SWDGE queues; ~3.1ns/idx instruction pacing on this
    silicon is the kernel's bottleneck - prepare_only/trigger_dma corrupts
    data on this ucode, and partition-offset matmuls crash the runtime, so
    neither is used). The idx stream is SBUF-resident (per-call idx DMAs
    throttled the gather by ~1.5x). Scaled one-hot blocks S_T[e,d] =
    vals[e]*(rowm[e]==d) are host-precomputed fp16, one block per
    (128-slot chunk x overlapping cell) with rows outside the cell zeroed, so
    chunks cross cell boundaries with full-128-partition matmuls only;
    TensorE accumulates AH feature-major in PSUM, 8 dest tiles (2 banks) per
    accumulation group; start/stop flags are tracked per 512-col half because
    PSUM start bits are per-bank. Dense-phase matmuls split at 512 cols (ISA
    limit on matmul output free elements)
  - AH is AllGathered (fp16 table) for hop 2; the same S_T stream is re-read
  - H2 = relu(W1.T@AH + W2.T@A2H), out = W_out.T@H2 feature-major fp16;
    b_out added on host

All 8 cores share one program: per-cell counts are the max over cores; each
core pads its slot stream with (idx=0, val=0) slots. Gather calls are rounded
to 128 slots so every gathered SBUF row is written (finite).
"""

import os
import sys

import numpy as np

if "/opt/trn_rl_repo" not in sys.path:
    sys.path.insert(0, "/opt/trn_rl_repo")

NCORES = 8
P = 128
FJ = 1024  # dense/PSUM-group column width (matmuls split at 512)
BANK_TILES = 8  # dest tiles per PSUM accumulation group (2 banks)
DMA_SCRATCH = 16384  # SWDGE ring (default size; gathers self-pace)


def _structure(row, col, vals, n_nodes):
    """Shared (cross-core) structure + per-core edge-stream arrays.

    Cells ((dest tile) x (src quartile)) hold the max-over-cores edge count
    rounded to 32 (PE partition-piece granularity), packed back-to-back into
    one gather call per (bank, quartile). Chunks are 128-slot windows of a
    call; a chunk may span cells, handled by 32-aligned partition-piece
    matmuls. S_T is an exact 0/1 one-hot in fp8 (vals are applied to the
    gathered chunk on-chip), so no quantization error on edge weights.
    """
    shard = n_nodes // NCORES
    shard_pad = -(-shard // P) * P
    npad = shard_pad * NCORES
    qrows = npad // 4
    assert qrows <= 32768, qrows
    T = shard_pad // P
    banks = [list(range(i, min(i + BANK_TILES, T))) for i in range(0, T, BANK_TILES)]

    core = row // shard
    r_loc = row - core * shard
    gcol = (col // shard) * shard_pad + (col % shard)
    q = gcol // qrows
    idxq = (gcol - q * qrows).astype(np.int16)
    t = r_loc // P
    rowm = (r_loc % P).astype(np.int64)

    ncells = T * 4
    cell = t * 4 + q
    cnt = np.zeros((NCORES, ncells), np.int64)
    for k in range(NCORES):
        m = core == k
        cnt[k] = np.bincount(cell[m], minlength=ncells)
    cell_sz = cnt.max(axis=0).copy()  # exact max-over-cores cell sizes
    # every tile must own >= 1 slot so its PSUM window gets written
    for tt in range(T):
        if cell_sz[tt * 4 : tt * 4 + 4].sum() == 0:
            cell_sz[tt * 4] = 1

    # stream order: for bank, for quartile, for tile-in-bank
    order_cells = np.array(
        [tt * 4 + qq for b in banks for qq in range(4) for tt in b]
    )
    ord_of_cell = np.empty(ncells, np.int64)
    ord_of_cell[order_cells] = np.arange(ncells)

    # calls: one per (bank, quartile); cells packed back-to-back inside, call
    # size rounded to 128 (all gather slots written -> finite data). Each
    # 128-slot chunk gets one matmul (full 128 partitions) per overlapping
    # cell; each matmul reads its own stream block whose rows outside the
    # cell's span are zero, so chunks may cross cell boundaries freely.
    calls = []
    cell_slot0 = np.zeros(ncells, np.int64)  # global stream slot of cell start
    pos = 0
    gchunk = 0  # gather-chunk counter (vals stream columns)
    sblock = 0  # stream-block counter (one-hot stream columns)
    for bank_i, b in enumerate(banks):
        for qq in range(4):
            cells = [(tt, int(cell_sz[tt * 4 + qq])) for tt in b]
            sz = sum(c for _, c in cells)
            if sz == 0:
                continue
            o = 0
            spans = []
            for tt, c in cells:
                if c:
                    cell_slot0[tt * 4 + qq] = pos + o
                    spans.append((tt, o, o + c))
                    o += c
            szr = -(-sz // 128) * 128
            nch = szr // 128
            ops = []  # (j, [(local_block, tile), ...])
            piece_spans = []  # per local block: (abs_lo, abs_hi) slot range
            for j in range(nch):
                lo, hi = j * 128, (j + 1) * 128
                pieces = []
                for tt, o0, o1 in spans:
                    a, bnd = max(o0, lo), min(o1, hi)
                    if a >= bnd:
                        continue
                    pieces.append((len(piece_spans), tt))
                    piece_spans.append((pos + a, pos + bnd))
                ops.append((j, pieces))
            calls.append(
                {
                    "bank": bank_i,
                    "qq": qq,
                    "off": pos,
                    "sz": szr,
                    "nch": nch,
                    "gc0": gchunk,
                    "sb0": sblock,
                    "nsb": len(piece_spans),
                    "piece_spans": piece_spans,
                    "ops": ops,
                }
            )
            pos += szr
            gchunk += nch
            sblock += len(piece_spans)
    S = pos
    nchunk = gchunk
    nsblock = sblock

    # slot -> (call base, gather-chunk base, stream block) maps
    call_off = np.zeros(S, np.int64)
    call_gc0 = np.zeros(S, np.int64)
    block_of_slot = np.full(S, -1, np.int64)
    for c in calls:
        call_off[c["off"] : c["off"] + c["sz"]] = c["off"]
        call_gc0[c["off"] : c["off"] + c["sz"]] = c["gc0"]
        for lb, (s0, s1) in enumerate(c["piece_spans"]):
            block_of_slot[s0:s1] = c["sb0"] + lb

    per_core = []
    for k in range(NCORES):
        m = core == k
        ek = ord_of_cell[cell[m]]
        perm = np.argsort(ek, kind="stable")
        sorted_ord = ek[perm]
        counts_in_order = cnt[k][order_cells]
        run_start = np.r_[0, np.cumsum(counts_in_order)[:-1]]
        rank = np.arange(len(sorted_ord)) - run_start[sorted_ord]
        slot = cell_slot0[order_cells[sorted_ord]] + rank
        idx_slots = np.zeros(S, np.int16)
        idx_slots[slot] = idxq[m][perm]
        # int16 index array: per call, wrap [sz] -> [16, sz/16]; replicate x8
        idx16 = np.zeros((16, S // 16), np.int16)
        for c in calls:
            off, sz = c["off"], c["sz"]
            idx16[:, off // 16 : (off + sz) // 16] = (
                idx_slots[off : off + sz].reshape(sz // 16, 16).T
            )
        idx16 = np.tile(idx16, (NCORES, 1))
        rel = slot - call_off[slot]
        part = rel % P
        chunk = call_gc0[slot] + rel // P
        # per-piece masked scaled one-hot blocks (fp16)
        st8 = np.zeros((nsblock, P, P), np.float16)
        st8[block_of_slot[slot], part, rowm[m][perm]] = vals[m][perm]
        st8 = np.ascontiguousarray(
            st8.transpose(1, 0, 2).reshape(P, nsblock * P)
        )
        per_core.append({"idx16": idx16, "st": st8})

    struct = {
        "shard": shard,
        "shard_pad": shard_pad,
        "npad": npad,
        "qrows": qrows,
        "T": T,
        "banks": banks,
        "calls": calls,
        "S": S,
        "nchunk": nchunk,
        "nsblock": nsblock,
    }
    return struct, per_core


def _np_f8():
    import ml_dtypes

    return ml_dtypes.float8_e4m3


def _build_nc(st):
    import concourse.mybir as mybir
    import concourse.tile as tile
    from concourse import bacc
    from concourse.masks import make_identity

    f32 = mybir.dt.float32
    f16 = mybir.dt.float16
    f8 = mybir.dt.float8e4
    i16 = mybir.dt.int16
    AF = mybir.ActivationFunctionType

    shard_pad, npad, qrows = st["shard_pad"], st["npad"], st["qrows"]
    banks, calls = st["banks"], st["calls"]
    S, nchunk, nsblock = st["S"], st["nchunk"], st["nsblock"]

    nc = bacc.Bacc(
        None,
        target_bir_lowering=False,
        num_swdge_queues=4,
        dynamic_dma_scratch_size=DMA_SCRATCH,
    )

    x_fm = nc.dram_tensor("x_fm", [P, shard_pad], f16, kind="ExternalInput")
    w_in = nc.dram_tensor("w_in", [P, P], f16, kind="ExternalInput")
    b_in = nc.dram_tensor("b_in", [P, 1], f32, kind="ExternalInput")
    w1 = nc.dram_tensor("w1", [P, P], f16, kind="ExternalInput")
    w2 = nc.dram_tensor("w2", [P, P], f16, kind="ExternalInput")
    w_out = nc.dram_tensor("w_out", [P, 1], f16, kind="ExternalInput")
    idx16_d = nc.dram_tensor("idx16", [P, S // 16], i16, kind="ExternalInput")
    st_d = nc.dram_tensor("st", [P, nsblock * P], f16, kind="ExternalInput")
    y_d = nc.dram_tensor("y", [1, shard_pad], f32, kind="ExternalOutput")
    cc_h_in = nc.dram_tensor("cc_h_in", [shard_pad, P], f16)
    h_tab = nc.dram_tensor("h_tab", [npad, P], f16, addr_space="Shared")
    cc_ah_in = nc.dram_tensor("cc_ah_in", [shard_pad, P], f16)
    ah_tab = nc.dram_tensor("ah_tab", [npad, P], f16, addr_space="Shared")
    rg = [list(range(NCORES))]

    gmax = max(-(-c["sz"] // P) * P for c in calls)  # gather tile cols (rounded)
    sbmax = max(c["nsb"] for c in calls) * P  # one-hot stream tile cols
    vmax = max(c["nch"] for c in calls)

    with tile.TileContext(nc) as tc:
        with (
            tc.tile_pool(name="const", bufs=1) as cp,
            tc.tile_pool(name="fm", bufs=1) as fmp,
            tc.tile_pool(name="xw", bufs=3) as xp,
            tc.tile_pool(name="hw", bufs=3) as hp,
            tc.tile_pool(name="nm", bufs=4) as nmp,
            tc.tile_pool(name="meta", bufs=1) as mp,
            tc.tile_pool(name="stb", bufs=3) as stp,
            tc.tile_pool(name="g", bufs=7) as gp,
            tc.tile_pool(name="ps_mm", bufs=2, space="PSUM") as pmm,
            tc.tile_pool(name="ps_tp", bufs=2, space="PSUM") as ptp,
            tc.tile_pool(name="ps_o", bufs=1, space="PSUM") as pso,
        ):
            t_ident = cp.tile([P, P], f16, tag="ident")
            make_identity(nc, t_ident[:])
            t_w_in = cp.tile([P, P], f16, tag="w_in")
            nc.sync.dma_start(out=t_w_in[:], in_=w_in[:])
            t_b_in = cp.tile([P, 1], f32, tag="b_in")
            nc.sync.dma_start(out=t_b_in[:], in_=b_in[:])
            t_w1 = cp.tile([P, P], f16, tag="w1")
            nc.sync.dma_start(out=t_w1[:], in_=w1[:])
            t_w2 = cp.tile([P, P], f16, tag="w2")
            nc.sync.dma_start(out=t_w2[:], in_=w2[:])
            t_wout = cp.tile([P, 1], f16, tag="wout")
            nc.sync.dma_start(out=t_wout[:], in_=w_out[:])
            t_idx = mp.tile([P, S // 16], i16, tag="idx")
            nc.sync.dma_start(out=t_idx[:], in_=idx16_d[:])
            ah_fm = fmp.tile([P, shard_pad], f16, tag="ah_fm")
            a2h_fm = fmp.tile([P, shard_pad], f16, tag="a2h_fm")

            # ---- H = relu(X @ W_in + b_in), feature-major; emit node-major table
            for j0 in range(0, shard_pad, FJ):
                w = min(FJ, shard_pad - j0)
                xt = xp.tile([P, FJ], f16, tag="x")
                nc.sync.dma_start(out=xt[:, :w], in_=x_fm[:, j0 : j0 + w])
                ps = pmm.tile([P, FJ], f32, tag="mm")
                for h0 in range(0, w, 512):
                    hw_ = min(512, w - h0)
                    nc.tensor.matmul(
                        out=ps[:, h0 : h0 + hw_],
                        lhsT=t_w_in[:],
                        rhs=xt[:, h0 : h0 + hw_],
                        start=True,
                        stop=True,
                    )
                ht = hp.tile([P, FJ], f16, tag="h")
                nc.scalar.activation(
                    ht[:, :w], ps[:, :w], AF.Relu, bias=t_b_in[:, :1], scale=1.0
                )
                for i0 in range(0, w, P):
                    pst = ptp.tile([P, P], f16, tag="tp")
                    nc.tensor.transpose(
                        out=pst[:], in_=ht[:, i0 : i0 + P], identity=t_ident[:]
                    )
                    nmt = nmp.tile([P, P], f16, tag="nm")
                    nc.scalar.copy(nmt[:], pst[:])
                    nc.sync.dma_start(
                        out=cc_h_in[j0 + i0 : j0 + i0 + P, :], in_=nmt[:]
                    )
            nc.gpsimd.collective_compute(
                "AllGather",
                mybir.AluOpType.bypass,
                replica_groups=rg,
                ins=[cc_h_in[:]],
                outs=[h_tab[:]],
            )

            # ---- SpMM pass over the edge stream
            state = {"ncall": 0}

            def spmm(src_tab, out_fm, nm_out):
                for bank_i, tiles in enumerate(banks):
                    ps = pmm.tile([P, FJ], f32, tag="mm")
                    bank_calls = [c for c in calls if c["bank"] == bank_i]
                    # start/stop are per PSUM bank: track each 512-col half
                    half_of = {tt: (tt - tiles[0]) // 4 for tt in tiles}
                    total_h = [0, 0]
                    for c in bank_calls:
                        for _, pieces in c["ops"]:
                            for _, tt in pieces:
                                total_h[half_of[tt]] += 1
                    done_h = [0, 0]
                    for c in bank_calls:
                        qq, off, sz = c["qq"], c["off"], c["sz"]
                        nch, szr = c["nch"], -(-c["sz"] // P) * P
                        g = gp.tile([P, gmax], f16, tag="g")
                        nc.gpsimd.dma_gather(
                            out_ap=g[:, :szr].rearrange("p (c d) -> p c d", d=P),
                            in_ap=src_tab[qq * qrows : (qq + 1) * qrows, :],
                            idxs_ap=t_idx[:, off // 16 : (off + sz) // 16],
                            num_idxs=sz,
                            num_idxs_reg=sz,
                            elem_size=P,
                            single_packet=False,
                            queue_num=state["ncall"] % 4,
                        )
                        state["ncall"] += 1
                        stt = stp.tile([P, sbmax], f16, tag="stb")
                        nc.scalar.dma_start(
                            out=stt[:, : c["nsb"] * P],
                            in_=st_d[:, c["sb0"] * P : (c["sb0"] + c["nsb"]) * P],
                        )
                        for j, pieces in c["ops"]:
                            for lb, tt in pieces:
                                ti = tt - tiles[0]
                                h = half_of[tt]
                                nc.tensor.matmul(
                                    out=ps[:, ti * P : (ti + 1) * P],
                                    lhsT=g[:, j * P : (j + 1) * P],
                                    rhs=stt[:, lb * P : (lb + 1) * P],
                                    start=(done_h[h] == 0),
                                    stop=(done_h[h] == total_h[h] - 1),
                                )
                                done_h[h] += 1
                    w = len(tiles) * P
                    f0 = tiles[0] * P
                    nc.scalar.copy(out_fm[:, f0 : f0 + w], ps[:, :w])
                    if nm_out is not None:
                        for tt in tiles:
                            pst = ptp.tile([P, P], f16, tag="tp")
                            nc.tensor.transpose(
                                out=pst[:],
                                in_=out_fm[:, tt * P : (tt + 1) * P],
                                identity=t_ident[:],
                            )
                            nmt = nmp.tile([P, P], f16, tag="nm")
                            nc.scalar.copy(nmt[:], pst[:])
                            nc.sync.dma_start(
                                out=nm_out[tt * P : (tt + 1) * P, :], in_=nmt[:]
                            )

            spmm(h_tab, ah_fm, cc_ah_in)
            nc.gpsimd.collective_compute(
                "AllGather",
                mybir.AluOpType.bypass,
                replica_groups=rg,
                ins=[cc_ah_in[:]],
                outs=[ah_tab[:]],
            )
            spmm(ah_tab, a2h_fm, None)

            # ---- H2 = relu(W1.T@AH + W2.T@A2H); y = W_out.T @ H2
            for j0 in range(0, shard_pad, FJ):
                w = min(FJ, shard_pad - j0)
                ps = pmm.tile([P, FJ], f32, tag="mm")
                for h0 in range(0, w, 512):
                    hw_ = min(512, w - h0)
                    nc.tensor.matmul(
                        out=ps[:, h0 : h0 + hw_],
                        lhsT=t_w1[:],
                        rhs=ah_fm[:, j0 + h0 : j0 + h0 + hw_],
                        start=True,
                        stop=False,
                    )
                    nc.tensor.matmul(
                        out=ps[:, h0 : h0 + hw_],
                        lhsT=t_w2[:],
                        rhs=a2h_fm[:, j0 + h0 : j0 + h0 + hw_],
                        start=False,
                        stop=True,
                    )
                h2 = hp.tile([P, FJ], f16, tag="h2")
                nc.scalar.activation(h2[:, :w], ps[:, :w], AF.Relu)
                ps2 = pso.tile([P, FJ], f32, tag="o")
                for h0 in range(0, w, 512):
                    hw_ = min(512, w - h0)
                    nc.tensor.matmul(
                        out=ps2[:1, h0 : h0 + hw_],
                        lhsT=t_wout[:, :1],
                        rhs=h2[:, h0 : h0 + hw_],
                        start=True,
                        stop=True,
                    )
                yt = nmp.tile([1, FJ], f32, tag="y")
                nc.scalar.copy(yt[:1, :w], ps2[:1, :w])
                nc.sync.dma_start(out=y_d[0:1, j0 : j0 + w], in_=yt[:1, :w])

    nc.finalize()
    return nc


def _make_in_maps(inputs, st, per_core):
    shard, shard_pad = st["shard"], st["shard_pad"]
    X = np.asarray(inputs["X"], np.float32).astype(np.float16)
    W_in = np.ascontiguousarray(np.asarray(inputs["W_in"], np.float32).astype(np.float16))
    b_in = np.asarray(inputs["b_in"], np.float32).reshape(P, 1)
    w1 = np.asarray(inputs["W_mp1"], np.float32).astype(np.float16)
    w2 = np.asarray(inputs["W_mp2"], np.float32).astype(np.float16)
    w_out = np.asarray(inputs["W_out"], np.float32).astype(np.float16).reshape(P, 1)
    in_maps = []
    for k in range(NCORES):
        x_fm = np.zeros((P, shard_pad), np.float16)
        x_fm[:, :shard] = X[k * shard : (k + 1) * shard].T
        in_maps.append(
            {
                "x_fm": x_fm,
                "w_in": W_in,
                "b_in": b_in,
                "w1": np.ascontiguousarray(w1),
                "w2": np.ascontiguousarray(w2),
                "w_out": np.ascontiguousarray(w_out),
                "idx16": per_core[k]["idx16"],
                "st": per_core[k]["st"],
            }
        )
    return in_maps


def kernel(**inputs):
    from concourse.bass_utils import run_bass_kernel_spmd

    row = np.asarray(inputs["row"], np.int64)
    col = np.asarray(inputs["col"], np.int64)
    vals = np.asarray(inputs["vals"], np.float32)
    n_nodes = int(np.asarray(inputs["X"]).shape[0])

    st, per_core = _structure(row, col, vals, n_nodes)
    nc = _build_nc(st)
    in_maps = _make_in_maps(inputs, st, per_core)

    trace = bool(int(os.environ.get("GNN_TRACE", "0")))
    res = run_bass_kernel_spmd(
        nc, in_maps, core_ids=list(range(NCORES)), trace=trace
    )
    if trace:
        kernel.last_exec_time_ns = res.exec_time_ns
        kernel.last_res = res

    b_out = float(np.asarray(inputs["b_out"]).reshape(-1)[0])
    shard = st["shard"]
    out = np.concatenate(
        [res.results[k]["y"][0, :shard] for k in range(NCORES)]
    ).astype(np.float32)
    return (out + b_out).reshape(n_nodes, 1)



# revision 47
# speedup vs baseline: 1.0340x; 1.0340x over previous
"""BetaGNN message-passing kernel for 8 Trainium2 NeuronCores.

Strategy (1D node partitioning):
  - nodes are sharded across 8 cores (12500 rows/core, padded to 12544 = 98*128)
  - H = relu(X @ W_in + b_in) per-shard feature-major (X, W_in in fp16), transposed on PE
    into a node-major fp16 gather table, AllGathered to all cores
  - SpMM (y[r] = sum vals[e]*H[col[e]] over edges with row[e]==r): edges are
    grouped by (dest-tile x src-quartile) cell with EXACT max-over-cores cell
    sizes (no per-cell 128-rounding); one gather call per (PSUM bank, quartile)
    fetches source rows via the GPSIMD dma_gather custom op (int16 indices over
    4 quarter-tables, 4 SWDGE queues; ~3.1ns/idx instruction pacing on this
    silicon is the kernel's bottleneck - prepare_only/trigger_dma corrupts
    data on this ucode, and partition-offset matmuls crash the runtime, so
    neither is used). The idx stream is SBUF-resident (per-call idx DMAs
    throttled the gather by ~1.5x). Scaled one-hot blocks S_T[e,d] =
    vals[e]*(rowm[e]==d) are host-precomputed fp16, one block per
    (128-slot chunk x overlapping cell) with rows outside the cell zeroed, so
    chunks cross cell boundaries with full-128-partition matmuls only;
    TensorE accumulates AH feature-major in PSUM, 8 dest tiles (2 banks) per
    accumulation group; start/stop flags are tracked per 512-col half because
    PSUM start bits are per-bank. Dense-phase matmuls split at 512 cols (ISA
    limit on matmul output free elements)
  - AH is AllGathered (fp16 table) for hop 2; the same S_T stream is re-read
  - H2 = relu(W1.T@AH + W2.T@A2H), out = W_out.T@H2 feature-major fp16;
    b_out added on host

All 8 cores share one program: per-cell counts are the max over cores; each
core pads its slot stream with (idx=0, val=0) slots. Gather calls are rounded
to 128 slots so every gathered SBUF row is written (finite).
"""

import os
import sys

import numpy as np

if "/opt/trn_rl_repo" not in sys.path:
    sys.path.insert(0, "/opt/trn_rl_repo")

NCORES = 8
P = 128
FJ = 1024  # dense/PSUM-group column width (matmuls split at 512)
BANK_TILES = 8  # dest tiles per PSUM accumulation group (2 banks)
DMA_SCRATCH = 16384  # SWDGE ring (default size; gathers self-pace)


def _structure(row, col, vals, n_nodes):
    """Shared (cross-core) structure + per-core edge-stream arrays.

    Cells ((dest tile) x (src quartile)) hold the max-over-cores edge count
    rounded to 32 (PE partition-piece granularity), packed back-to-back into
    one gather call per (bank, quartile). Chunks are 128-slot windows of a
    call; a chunk may span cells, handled by 32-aligned partition-piece
    matmuls. S_T is an exact 0/1 one-hot in fp8 (vals are applied to the
    gathered chunk on-chip), so no quantization error on edge weights.
    """
    shard = n_nodes // NCORES
    shard_pad = -(-shard // P) * P
    npad = shard_pad * NCORES
    qrows = npad // 4
    assert qrows <= 32768, qrows
    T = shard_pad // P
    banks = [list(range(i, min(i + BANK_TILES, T))) for i in range(0, T, BANK_TILES)]

    core = row // shard
    r_loc = row - core * shard
    gcol = (col // shard) * shard_pad + (col % shard)
    q = gcol // qrows
    idxq = (gcol - q * qrows).astype(np.int16)
    t = r_loc // P
    rowm = (r_loc % P).astype(np.int64)

    ncells = T * 4
    cell = t * 4 + q
    cnt = np.zeros((NCORES, ncells), np.int64)
    for k in range(NCORES):
        m = core == k
        cnt[k] = np.bincount(cell[m], minlength=ncells)
    cell_sz = cnt.max(axis=0).copy()  # exact max-over-cores cell sizes
    # every tile must own >= 1 slot so its PSUM window gets written
    for tt in range(T):
        if cell_sz[tt * 4 : tt * 4 + 4].sum() == 0:
            cell_sz[tt * 4] = 1

    # stream order: for bank, for quartile, for tile-in-bank
    order_cells = np.array(
        [tt * 4 + qq for b in banks for qq in range(4) for tt in b]
    )
    ord_of_cell = np.empty(ncells, np.int64)
    ord_of_cell[order_cells] = np.arange(ncells)

    # calls: one per (bank, quartile); cells packed back-to-back inside, call
    # size rounded to 128 (all gather slots written -> finite data). Each
    # 128-slot chunk gets one matmul (full 128 partitions) per overlapping
    # cell; each matmul reads its own stream block whose rows outside the
    # cell's span are zero, so chunks may cross cell boundaries freely.
    calls = []
    cell_slot0 = np.zeros(ncells, np.int64)  # global stream slot of cell start
    pos = 0
    gchunk = 0  # gather-chunk counter (vals stream columns)
    sblock = 0  # stream-block counter (one-hot stream columns)
    for bank_i, b in enumerate(banks):
        for qq in range(4):
            cells = [(tt, int(cell_sz[tt * 4 + qq])) for tt in b]
            sz = sum(c for _, c in cells)
            if sz == 0:
                continue
            o = 0
            spans = []
            for tt, c in cells:
                if c:
                    cell_slot0[tt * 4 + qq] = pos + o
                    spans.append((tt, o, o + c))
                    o += c
            szr = -(-sz // 128) * 128
            nch = szr // 128
            ops = []  # (j, [(local_block, tile), ...])
            piece_spans = []  # per local block: (abs_lo, abs_hi) slot range
            for j in range(nch):
                lo, hi = j * 128, (j + 1) * 128
                pieces = []
                for tt, o0, o1 in spans:
                    a, bnd = max(o0, lo), min(o1, hi)
                    if a >= bnd:
                        continue
                    pieces.append((len(piece_spans), tt))
                    piece_spans.append((pos + a, pos + bnd))
                ops.append((j, pieces))
            calls.append(
                {
                    "bank": bank_i,
                    "qq": qq,
                    "off": pos,
                    "sz": szr,
                    "nch": nch,
                    "gc0": gchunk,
                    "sb0": sblock,
                    "nsb": len(piece_spans),
                    "piece_spans": piece_spans,
                    "ops": ops,
                }
            )
            pos += szr
            gchunk += nch
            sblock += len(piece_spans)
    S = pos
    nchunk = gchunk
    nsblock = sblock

    # slot -> (call base, gather-chunk base, stream block) maps
    call_off = np.zeros(S, np.int64)
    call_gc0 = np.zeros(S, np.int64)
    block_of_slot = np.full(S, -1, np.int64)
    for c in calls:
        call_off[c["off"] : c["off"] + c["sz"]] = c["off"]
        call_gc0[c["off"] : c["off"] + c["sz"]] = c["gc0"]
        for lb, (s0, s1) in enumerate(c["piece_spans"]):
            block_of_slot[s0:s1] = c["sb0"] + lb

    per_core = []
    for k in range(NCORES):
        m = core == k
        ek = ord_of_cell[cell[m]]
        perm = np.argsort(ek, kind="stable")
        sorted_ord = ek[perm]
        counts_in_order = cnt[k][order_cells]
        run_start = np.r_[0, np.cumsum(counts_in_order)[:-1]]
        rank = np.arange(len(sorted_ord)) - run_start[sorted_ord]
        slot = cell_slot0[order_cells[sorted_ord]] + rank
        idx_slots = np.zeros(S, np.int16)
        idx_slots[slot] = idxq[m][perm]
        # int16 index array: per call, wrap [sz] -> [16, sz/16]; replicate x8
        idx16 = np.zeros((16, S // 16), np.int16)
        for c in calls:
            off, sz = c["off"], c["sz"]
            idx16[:, off // 16 : (off + sz) // 16] = (
                idx_slots[off : off + sz].reshape(sz // 16, 16).T
            )
        idx16 = np.tile(idx16, (NCORES, 1))
        rel = slot - call_off[slot]
        part = rel % P
        chunk = call_gc0[slot] + rel // P
        # per-piece masked scaled one-hot blocks (fp16)
        st8 = np.zeros((nsblock, P, P), np.float16)
        st8[block_of_slot[slot], part, rowm[m][perm]] = vals[m][perm]
        st8 = np.ascontiguousarray(
            st8.transpose(1, 0, 2).reshape(P, nsblock * P)
        )
        per_core.append({"idx16": idx16, "st": st8})

    struct = {
        "shard": shard,
        "shard_pad": shard_pad,
        "npad": npad,
        "qrows": qrows,
        "T": T,
        "banks": banks,
        "calls": calls,
        "S": S,
        "nchunk": nchunk,
        "nsblock": nsblock,
    }
    return struct, per_core


def _np_f8():
    import ml_dtypes

    return ml_dtypes.float8_e4m3


def _build_nc(st):
    import concourse.mybir as mybir
    import concourse.tile as tile
    from concourse import bacc
    from concourse.masks import make_identity

    f32 = mybir.dt.float32
    f16 = mybir.dt.float16
    f8 = mybir.dt.float8e4
    i16 = mybir.dt.int16
    AF = mybir.ActivationFunctionType

    shard_pad, npad, qrows = st["shard_pad"], st["npad"], st["qrows"]
    banks, calls = st["banks"], st["calls"]
    S, nchunk, nsblock = st["S"], st["nchunk"], st["nsblock"]

    nc = bacc.Bacc(
        None,
        target_bir_lowering=False,
        num_swdge_queues=4,
        dynamic_dma_scratch_size=DMA_SCRATCH,
    )

    x_fm = nc.dram_tensor("x_fm", [P, shard_pad], f16, kind="ExternalInput")
    w_in = nc.dram_tensor("w_in", [P, P], f16, kind="ExternalInput")
    b_in = nc.dram_tensor("b_in", [P, 1], f32, kind="ExternalInput")
    w1 = nc.dram_tensor("w1", [P, P], f16, kind="ExternalInput")
    w2 = nc.dram_tensor("w2", [P, P], f16, kind="ExternalInput")
    w_out = nc.dram_tensor("w_out", [P, 1], f16, kind="ExternalInput")
    idx16_d = nc.dram_tensor("idx16", [P, S // 16], i16, kind="ExternalInput")
    st_d = nc.dram_tensor("st", [P, nsblock * P], f16, kind="ExternalInput")
    y_d = nc.dram_tensor("y", [1, shard_pad], f32, kind="ExternalOutput")
    cc_h_in = nc.dram_tensor("cc_h_in", [shard_pad, P], f16)
    h_tab = nc.dram_tensor("h_tab", [npad, P], f16, addr_space="Shared")
    cc_ah_in = nc.dram_tensor("cc_ah_in", [shard_pad, P], f16)
    ah_tab = nc.dram_tensor("ah_tab", [npad, P], f16, addr_space="Shared")
    rg = [list(range(NCORES))]

    gmax = max(-(-c["sz"] // P) * P for c in calls)  # gather tile cols (rounded)
    sbmax = max(c["nsb"] for c in calls) * P  # one-hot stream tile cols
    vmax = max(c["nch"] for c in calls)

    with tile.TileContext(nc) as tc:
        with (
            tc.tile_pool(name="const", bufs=1) as cp,
            tc.tile_pool(name="fm", bufs=1) as fmp,
            tc.tile_pool(name="xw", bufs=3) as xp,
            tc.tile_pool(name="hw", bufs=3) as hp,
            tc.tile_pool(name="nm", bufs=4) as nmp,
            tc.tile_pool(name="meta", bufs=1) as mp,
            tc.tile_pool(name="stb", bufs=3) as stp,
            tc.tile_pool(name="g", bufs=6) as gp,
            tc.tile_pool(name="ps_mm", bufs=2, space="PSUM") as pmm,
            tc.tile_pool(name="ps_tp", bufs=2, space="PSUM") as ptp,
            tc.tile_pool(name="ps_o", bufs=1, space="PSUM") as pso,
        ):
            t_ident = cp.tile([P, P], f16, tag="ident")
            make_identity(nc, t_ident[:])
            t_w_in = cp.tile([P, P], f16, tag="w_in")
            nc.sync.dma_start(out=t_w_in[:], in_=w_in[:])
            t_b_in = cp.tile([P, 1], f32, tag="b_in")
            nc.sync.dma_start(out=t_b_in[:], in_=b_in[:])
            t_w1 = cp.tile([P, P], f16, tag="w1")
            nc.sync.dma_start(out=t_w1[:], in_=w1[:])
            t_w2 = cp.tile([P, P], f16, tag="w2")
            nc.sync.dma_start(out=t_w2[:], in_=w2[:])
            t_wout = cp.tile([P, 1], f16, tag="wout")
            nc.sync.dma_start(out=t_wout[:], in_=w_out[:])
            t_idx = mp.tile([P, S // 16], i16, tag="idx")
            nc.sync.dma_start(out=t_idx[:], in_=idx16_d[:])
            ah_fm = fmp.tile([P, shard_pad], f16, tag="ah_fm")
            a2h_fm = fmp.tile([P, shard_pad], f16, tag="a2h_fm")

            # ---- H = relu(X @ W_in + b_in), feature-major; emit node-major table
            for j0 in range(0, shard_pad, FJ):
                w = min(FJ, shard_pad - j0)
                xt = xp.tile([P, FJ], f16, tag="x")
                nc.sync.dma_start(out=xt[:, :w], in_=x_fm[:, j0 : j0 + w])
                ps = pmm.tile([P, FJ], f32, tag="mm")
                for h0 in range(0, w, 512):
                    hw_ = min(512, w - h0)
                    nc.tensor.matmul(
                        out=ps[:, h0 : h0 + hw_],
                        lhsT=t_w_in[:],
                        rhs=xt[:, h0 : h0 + hw_],
                        start=True,
                        stop=True,
                    )
                ht = hp.tile([P, FJ], f16, tag="h")
                nc.scalar.activation(
                    ht[:, :w], ps[:, :w], AF.Relu, bias=t_b_in[:, :1], scale=1.0
                )
                for i0 in range(0, w, P):
                    pst = ptp.tile([P, P], f16, tag="tp")
                    nc.tensor.transpose(
                        out=pst[:], in_=ht[:, i0 : i0 + P], identity=t_ident[:]
                    )
                    nmt = nmp.tile([P, P], f16, tag="nm")
                    nc.scalar.copy(nmt[:], pst[:])
                    nc.sync.dma_start(
                        out=cc_h_in[j0 + i0 : j0 + i0 + P, :], in_=nmt[:]
                    )
            nc.gpsimd.collective_compute(
                "AllGather",
                mybir.AluOpType.bypass,
                replica_groups=rg,
                ins=[cc_h_in[:]],
                outs=[h_tab[:]],
            )

            # ---- SpMM pass over the edge stream
            state = {"ncall": 0}

            def spmm(src_tab, out_fm, nm_out):
                for bank_i, tiles in enumerate(banks):
                    ps = pmm.tile([P, FJ], f32, tag="mm")
                    bank_calls = [c for c in calls if c["bank"] == bank_i]
                    # start/stop are per PSUM bank: track each 512-col half
                    half_of = {tt: (tt - tiles[0]) // 4 for tt in tiles}
                    total_h = [0, 0]
                    for c in bank_calls:
                        for _, pieces in c["ops"]:
                            for _, tt in pieces:
                                total_h[half_of[tt]] += 1
                    done_h = [0, 0]
                    for c in bank_calls:
                        qq, off, sz = c["qq"], c["off"], c["sz"]
                        nch, szr = c["nch"], -(-c["sz"] // P) * P
                        g = gp.tile([P, gmax], f16, tag="g")
                        nc.gpsimd.dma_gather(
                            out_ap=g[:, :szr].rearrange("p (c d) -> p c d", d=P),
                            in_ap=src_tab[qq * qrows : (qq + 1) * qrows, :],
                            idxs_ap=t_idx[:, off // 16 : (off + sz) // 16],
                            num_idxs=sz,
                            num_idxs_reg=sz,
                            elem_size=P,
                            single_packet=False,
                            queue_num=state["ncall"] % 4,
                        )
                        state["ncall"] += 1
                        stt = stp.tile([P, sbmax], f16, tag="stb")
                        nc.scalar.dma_start(
                            out=stt[:, : c["nsb"] * P],
                            in_=st_d[:, c["sb0"] * P : (c["sb0"] + c["nsb"]) * P],
                        )
                        for j, pieces in c["ops"]:
                            for lb, tt in pieces:
                                ti = tt - tiles[0]
                                h = half_of[tt]
                                nc.tensor.matmul(
                                    out=ps[:, ti * P : (ti + 1) * P],
                                    lhsT=g[:, j * P : (j + 1) * P],
                                    rhs=stt[:, lb * P : (lb + 1) * P],
                                    start=(done_h[h] == 0),
                                    stop=(done_h[h] == total_h[h] - 1),
                                )
                                done_h[h] += 1
                    w = len(tiles) * P
                    f0 = tiles[0] * P
                    nc.scalar.copy(out_fm[:, f0 : f0 + w], ps[:, :w])
                    if nm_out is not None:
                        for tt in tiles:
                            pst = ptp.tile([P, P], f16, tag="tp")
                            nc.tensor.transpose(
                                out=pst[:],
                                in_=out_fm[:, tt * P : (tt + 1) * P],
                                identity=t_ident[:],
                            )
                            nmt = nmp.tile([P, P], f16, tag="nm")
                            nc.scalar.copy(nmt[:], pst[:])
                            nc.sync.dma_start(
                                out=nm_out[tt * P : (tt + 1) * P, :], in_=nmt[:]
                            )

            spmm(h_tab, ah_fm, cc_ah_in)
            nc.gpsimd.collective_compute(
                "AllGather",
                mybir.AluOpType.bypass,
                replica_groups=rg,
                ins=[cc_ah_in[:]],
                outs=[ah_tab[:]],
            )
            spmm(ah_tab, a2h_fm, None)

            # ---- H2 = relu(W1.T@AH + W2.T@A2H); y = W_out.T @ H2
            for j0 in range(0, shard_pad, FJ):
                w = min(FJ, shard_pad - j0)
                ps = pmm.tile([P, FJ], f32, tag="mm")
                for h0 in range(0, w, 512):
                    hw_ = min(512, w - h0)
                    nc.tensor.matmul(
                        out=ps[:, h0 : h0 + hw_],
                        lhsT=t_w1[:],
                        rhs=ah_fm[:, j0 + h0 : j0 + h0 + hw_],
                        start=True,
                        stop=False,
                    )
                    nc.tensor.matmul(
                        out=ps[:, h0 : h0 + hw_],
                        lhsT=t_w2[:],
                        rhs=a2h_fm[:, j0 + h0 : j0 + h0 + hw_],
                        start=False,
                        stop=True,
                    )
                h2 = hp.tile([P, FJ], f16, tag="h2")
                nc.scalar.activation(h2[:, :w], ps[:, :w], AF.Relu)
                ps2 = pso.tile([P, FJ], f32, tag="o")
                for h0 in range(0, w, 512):
                    hw_ = min(512, w - h0)
                    nc.tensor.matmul(
                        out=ps2[:1, h0 : h0 + hw_],
                        lhsT=t_wout[:, :1],
                        rhs=h2[:, h0 : h0 + hw_],
                        start=True,
                        stop=True,
                    )
                yt = nmp.tile([1, FJ], f32, tag="y")
                nc.scalar.copy(yt[:1, :w], ps2[:1, :w])
                nc.sync.dma_start(out=y_d[0:1, j0 : j0 + w], in_=yt[:1, :w])

    nc.finalize()
    return nc


def _make_in_maps(inputs, st, per_core):
    shard, shard_pad = st["shard"], st["shard_pad"]
    X = np.asarray(inputs["X"], np.float32).astype(np.float16)
    W_in = np.ascontiguousarray(np.asarray(inputs["W_in"], np.float32).astype(np.float16))
    b_in = np.asarray(inputs["b_in"], np.float32).reshape(P, 1)
    w1 = np.asarray(inputs["W_mp1"], np.float32).astype(np.float16)
    w2 = np.asarray(inputs["W_mp2"], np.float32).astype(np.float16)
    w_out = np.asarray(inputs["W_out"], np.float32).astype(np.float16).reshape(P, 1)
    in_maps = []
    for k in range(NCORES):
        x_fm = np.zeros((P, shard_pad), np.float16)
        x_fm[:, :shard] = X[k * shard : (k + 1) * shard].T
        in_maps.append(
            {
                "x_fm": x_fm,
                "w_in": W_in,
                "b_in": b_in,
                "w1": np.ascontiguousarray(w1),
                "w2": np.ascontiguousarray(w2),
                "w_out": np.ascontiguousarray(w_out),
                "idx16": per_core[k]["idx16"],
                "st": per_core[k]["st"],
            }
        )
    return in_maps


def kernel(**inputs):
    from concourse.bass_utils import run_bass_kernel_spmd

    row = np.asarray(inputs["row"], np.int64)
    col = np.asarray(inputs["col"], np.int64)
    vals = np.asarray(inputs["vals"], np.float32)
    n_nodes = int(np.asarray(inputs["X"]).shape[0])

    st, per_core = _structure(row, col, vals, n_nodes)
    nc = _build_nc(st)
    in_maps = _make_in_maps(inputs, st, per_core)

    trace = bool(int(os.environ.get("GNN_TRACE", "0")))
    res = run_bass_kernel_spmd(
        nc, in_maps, core_ids=list(range(NCORES)), trace=trace
    )
    if trace:
        kernel.last_exec_time_ns = res.exec_time_ns
        kernel.last_res = res

    b_out = float(np.asarray(inputs["b_out"]).reshape(-1)[0])
    shard = st["shard"]
    out = np.concatenate(
        [res.results[k]["y"][0, :shard] for k in range(NCORES)]
    ).astype(np.float32)
    return (out + b_out).reshape(n_nodes, 1)

